# revision 29
# baseline (speedup 1.0000x reference)
"""Trainium2 Bass kernel for the non-local attention denoising block.

Computation (per batch b of the [2, 3, 96, 96] input):
    x      = input[b].reshape(3, 9216)                  # [C, N]
    S      = x^T x / sqrt(3)                            # [N, N] never materialized
    A      = softmax(S, axis=1)
    f      = (A @ x^T)^T                                # [C, N]
    out[b] = input[b] + conv3x3_same(f) + conv_b

Sharding: 8 cores = 2 batches x 4 query bands of 24 image rows. Each core
computes its band (+1 halo image row each side so the 3x3 conv needs no
cross-core exchange) against the full key sequence, flash-attention style:

  - mm1: S^T tile [128 k-part, 512 q-free] = matmul(lhsT=xk block, rhs=xq
    chunk) in float32r. The C=3 contraction is zero-padded to K=128 because
    the PE streams at half rate (427ns vs 216/313ns per 512 cols) whenever
    the moving operand has fewer than 128 partitions.
  - exp on the scalar engine, 3 k-blocks per instruction (free dim 1536)
    to amortize the ~352-cycle ACT instruction overhead; output bf16.
  - mm2: U^T[4, q] += matmul(lhsT=v4[kb] ([128, 4] = x^T with an appended
    ones column so row 3 accumulates the softmax denominator Z), rhs=e^T).
    The three k-blocks of a group go to three PE column groups (PSUM rows
    32j..32j+3) so their drains proceed concurrently (M=4 matmuls are
    drain-bound at 427ns otherwise).
  - softmax divide: PE-transpose the three U^T partials into one [128, 4]
    (transpose-and-sum via accumulating matmuls against a replicated I4),
    DVE reciprocal + per-partition tensor_scalar multiply, PE-transpose
    back. (Compute engines cannot address partition bases other than
    0 mod 32, so all cross-partition movement goes through the PE.)
  - conv: F goes into a width-98 zero-padded row layout (wrap columns and
    out-of-image rows read zeros = SAME padding), 9 shifted SBUF->SBUF DMA
    copies build a [27, 2352] stack, and the whole 3x3 conv is 5 K=27
    matmuls. Residual + bias are added in the padded layout; the final DMA
    strides the padding away.

No [N, N] tensor ever touches HBM; per-core HBM traffic is ~6 MB (dominated
by the zero-padded xk/xq loads, which overlap compute).
"""

import math
import os
import sys

for _p in (
    "/opt/trn_rl_repo",
    "/root/.axon_site",
    "/root/.axon_site/_ro/trn_rl_repo",
    "/root/.axon_site/_ro/pypackages",
):
    if os.path.isdir(_p) and _p not in sys.path:
        sys.path.append(_p)

import ml_dtypes  # noqa: E402
import numpy as np  # noqa: E402

import concourse.bacc as bacc  # noqa: E402
import concourse.bass as bass  # noqa: E402
import concourse.tile as tile  # noqa: E402
from concourse import mybir  # noqa: E402
from concourse.bass_utils import run_bass_kernel_spmd  # noqa: E402

# Problem shape (hardcoded per the harness contract).
B, C, H, W = 2, 3, 96, 96
N = H * W                      # 9216 spatial positions (keys)
BANDS = 4                      # query bands per batch
BAND_ROWS = H // BANDS         # 24 image rows per band
BAND_Q = BAND_ROWS * W         # 2304 output queries per core
HALO_ROWS = BAND_ROWS + 2      # +1 halo row each side for the conv
QN = HALO_ROWS * W             # 2496 computed queries per core
QPAD = QN                      # 2496 = 4 x 512 + 448 matmul chunks
KB = N // 128                  # 72 key blocks of 128
KB_GRP = 3                     # k-blocks fused per exp instruction
QCHUNK = 512
NQC = 5                        # 4 x 512 + 448
WP = W + 2                     # padded image row width for the conv
CONVN = BAND_ROWS * WP         # 2352 conv output positions (padded layout)
FPN = HALO_ROWS * WP + 4       # F_p length (+4 slack for shift reads)
INV_SQRT_C = 1.0 / math.sqrt(C)

F32 = mybir.dt.float32
F32R = mybir.dt.float32r
BF16 = mybir.dt.bfloat16
EXP = mybir.ActivationFunctionType.Exp

SHIFTS = [(dy, dx) for dy in (-1, 0, 1) for dx in (-1, 0, 1)]
MM1_FLAT = True            # K=128 zero-padded mm1 (K<128 slices miscompute)
TRS_DT_ENV = False         # f32 transposes (f32r transposes crash at runtime)


def build_nc() -> bass.Bass:
    nc = bacc.Bacc()

    xk = nc.declare_dram_parameter("xk", [128, N], F32R, isOutput=False)
    xq = nc.declare_dram_parameter("xq", [128, QPAD], F32R, isOutput=False)
    v4 = nc.declare_dram_parameter("v4", [128, KB * 4], BF16, isOutput=False)
    mask = nc.declare_dram_parameter("mask", [C, QPAD], F32, isOutput=False)
    resid = nc.declare_dram_parameter("resid", [C, CONVN], F32, isOutput=False)
    wmat = nc.declare_dram_parameter("wmat", [27, C], BF16, isOutput=False)
    TRS = F32R if TRS_DT_ENV else F32
    eye4x = nc.declare_dram_parameter("eye4x", [128, 4], TRS, isOutput=False)
    id128 = nc.declare_dram_parameter("id128", [128, 128], TRS, isOutput=False)
    out = nc.declare_dram_parameter("out", [C, BAND_Q], F32, isOutput=True)

    NXKC = 8                     # xk arrives in 8 chunk DMAs
    XKC = N // NXKC

    with tile.TileContext(nc) as tc, \
            tc.tile_pool(name="persist", bufs=1) as P, \
            tc.tile_pool(name="work", bufs=2) as WK, \
            tc.tile_pool(name="small", bufs=3) as SM, \
            tc.tile_pool(name="epool", bufs=3) as EPl, \
            tc.tile_pool(name="spool", bufs=2, space="PSUM") as SP, \
            tc.tile_pool(name="upool", bufs=1, space="PSUM") as UP, \
            tc.tile_pool(name="tpool", bufs=1, space="PSUM") as TP:

        # ---- load inputs -------------------------------------------------
        xq_sb = P.tile([128, QPAD], F32R, tag="xq", name="xq_sb")
        xk_sb = P.tile([128, N], F32R, tag="xk", name="xk_sb")
        v4_sb = P.tile([128, KB * 4], BF16, tag="v4", name="v4_sb")
        for c0 in range(0, QPAD, QCHUNK):
            cn_ = min(QCHUNK, QPAD - c0)
            nc.sync.dma_start(out=xq_sb[:, c0:c0 + cn_],
                              in_=xq[:, c0:c0 + cn_])
        # small first chunk so the first k-groups start sooner
        xk_edges = [0, 384, 768, 1152, 1536, 2304, 3456, 4608, 6912, N]
        for a_, b_ in zip(xk_edges, xk_edges[1:]):
            if b_ > a_:
                nc.sync.dma_start(out=xk_sb[:, a_:b_], in_=xk[:, a_:b_])
        nc.sync.dma_start(out=v4_sb, in_=v4[:])
        mask_sb = P.tile([C, QPAD], F32, tag="mask", name="mask_sb")
        nc.sync.dma_start(out=mask_sb, in_=mask[:])
        resid_sb = P.tile([C, CONVN], F32, tag="resid", name="resid_sb")
        nc.sync.dma_start(out=resid_sb, in_=resid[:])
        wmat_sb = P.tile([27, C], BF16, tag="wmat", name="wmat_sb")
        nc.sync.dma_start(out=wmat_sb, in_=wmat[:])
        eye4x_sb = P.tile([128, 4], TRS, tag="eye4x", name="eye4x_sb")
        nc.sync.dma_start(out=eye4x_sb, in_=eye4x[:])
        id128_sb = P.tile([128, 128], TRS, tag="id128", name="id128_sb")
        nc.sync.dma_start(out=id128_sb, in_=id128[:])

        F_sb = P.tile([C, QPAD], F32, tag="F", name="F_sb")
        F_m = P.tile([C, QPAD], F32, tag="Fm", name="F_m")
        F_p = P.tile([C, FPN], BF16, tag="Fp", name="F_p")
        nc.vector.memset(F_p, 0.0)
        F_sh = P.tile([27, CONVN], BF16, tag="Fsh", name="F_sh")
        nc.vector.memset(F_sh, 0.0)
        out_pad = P.tile([C, CONVN], F32, tag="opad", name="out_pad")
        F_p_r = F_p[:, 0:HALO_ROWS * WP].rearrange("p (r w) -> p r w", w=WP)
        F_m_r = F_m[:, 0:QN].rearrange("p (r w) -> p r w", w=W)
        pstate = {"rows": 0, "cols": 0, "rc": 0}
        CONV_CH = 392

        def emit_pipeline(rows_now):
            if rows_now > pstate["rows"]:
                nc.vector.tensor_copy(
                    F_p_r[:, pstate["rows"]:rows_now, 1:1 + W],
                    F_m_r[:, pstate["rows"]:rows_now, :])
                pstate["rows"] = rows_now
            cols_now = max(0, min(CONVN, rows_now * WP - 197))
            if rows_now == HALO_ROWS:
                cols_now = CONVN
            if cols_now > pstate["cols"]:
                for s, (dy, dx) in enumerate(SHIFTS):
                    off = WP + dy * WP + dx
                    a = max(pstate["cols"], -off)
                    if cols_now > a:
                        nc.gpsimd.dma_start(
                            out=F_sh[3 * s:3 * s + 3, a:cols_now],
                            in_=F_p[:, a + off:cols_now + off])
                pstate["cols"] = cols_now
            while pstate["rc"] < CONVN // CONV_CH and                     (pstate["rc"] + 1) * CONV_CH <= pstate["cols"]:
                c0 = pstate["rc"] * CONV_CH
                cv_ps = TP.tile([C, CONV_CH], F32, tag="T", name="cv_ps",
                                padded_shape=[C, QCHUNK])
                nc.tensor.matmul(cv_ps, lhsT=wmat_sb,
                                 rhs=F_sh[:, c0:c0 + CONV_CH],
                                 start=True, stop=True)
                nc.vector.tensor_add(out_pad[:, c0:c0 + CONV_CH], cv_ps,
                                     resid_sb[:, c0:c0 + CONV_CH])
                pstate["rc"] += 1
        # ---- flash attention main loop ----------------------------------
        for qc in range(NQC):
            q0 = qc * QCHUNK
            qn = min(QCHUNK, QPAD - q0)
            U_ps = UP.tile([128, qn], F32, tag="U", name="U_ps",
                           padded_shape=[128, QCHUNK])
            for t in range(KB // KB_GRP):
                S_ps = SP.tile([128, KB_GRP, qn], F32, tag="S", name="S_ps",
                               padded_shape=[128, KB_GRP, QCHUNK])
                for j in range(KB_GRP):
                    kb = t * KB_GRP + j
                    nc.tensor.matmul(
                        S_ps[:, j, :],
                        lhsT=xk_sb[:, kb * 128:(kb + 1) * 128] if MM1_FLAT
                        else xk_sb[32 * j:32 * j + C, kb * 128:(kb + 1) * 128],
                        rhs=xq_sb[:, q0:q0 + qn] if MM1_FLAT
                        else xq_sb[32 * j:32 * j + C, q0:q0 + qn],
                        start=True, stop=True,
                    )
                e_sb = EPl.tile([128, KB_GRP, qn], BF16, tag="e", name="e_sb",
                                padded_shape=[128, KB_GRP, QCHUNK])
                nc.scalar.activation(e_sb, S_ps, EXP, scale=INV_SQRT_C)
                for j in range(KB_GRP):
                    kb = t * KB_GRP + j
                    nc.tensor.matmul(
                        U_ps[32 * j:32 * j + 4, 0:qn],
                        lhsT=v4_sb[:, kb * 4:kb * 4 + 4],
                        rhs=e_sb[:, j, :],
                        start=(t == 0), stop=(t == KB // KB_GRP - 1),
                        skip_group_check=True,
                    )

            # softmax division via PE transposes (partition-aligned only)
            U_sb = WK.tile([128, qn], TRS, tag="Usb", name="U_sb",
                           padded_shape=[128, QCHUNK])
            nc.vector.tensor_copy(U_sb, U_ps)
            for c in range((qn + 127) // 128):
                cw = min(128, qn - c * 128)
                # last qc: main loop is done, the S slots are free -- use
                # them so the exposed tail divide double-buffers and the
                # TP slot stays free for the conv chunks
                TPt = (SP if qc == NQC - 1 else TP).tile(
                    [128, 132], F32,
                    tag="S" if qc == NQC - 1 else "T", name="TPt",
                    padded_shape=None if qc == NQC - 1 else [128, QCHUNK])
                Ut = TPt[0:cw, 0:4]
                # transpose-and-sum the three column-group partials
                for j in range(KB_GRP):
                    nc.tensor.matmul(
                        Ut,
                        lhsT=U_sb[32 * j:32 * j + 4, c * 128:c * 128 + cw],
                        rhs=eye4x_sb[32 * j:32 * j + 4, :],
                        start=(j == 0), stop=(j == KB_GRP - 1),
                        skip_group_check=True,
                    )
                r_sb = SM.tile([128, 1], F32, tag="r", name="r_sb")
                nc.vector.reciprocal(r_sb[0:cw, :], Ut[:, 3:4])
                Ft_sb = SM.tile([128, 4], TRS, tag="Ft", name="Ft_sb")
                nc.vector.tensor_scalar_mul(Ft_sb[0:cw, :], Ut, r_sb[0:cw, :])
                Fb = TPt[0:4, 4:4 + cw]
                nc.tensor.matmul(Fb, lhsT=Ft_sb[0:cw, :],
                                 rhs=id128_sb[0:cw, 0:cw],
                                 start=True, stop=True)
                nc.vector.tensor_copy(F_sb[:, q0 + c * 128:q0 + c * 128 + cw],
                                      Fb[0:3, :])
            nc.vector.tensor_mul(F_m[:, q0:q0 + qn],
                                 F_sb[:, q0:q0 + qn],
                                 mask_sb[:, q0:q0 + qn])
            if qc >= 2:
                emit_pipeline(min(HALO_ROWS, (q0 + qn) // W))


        # ---- tail: (conv emitted per-qc above) ---------------------------
        out_pad_r = out_pad.rearrange("p (r w) -> p r w", w=WP)
        nc.sync.dma_start(out=out[:].rearrange("p (r w) -> p r w", w=W),
                          in_=out_pad_r[:, :, 1:1 + W])

    nc.compile()
    return nc


_CACHE: dict = {}


def _get_nc() -> bass.Bass:
    if "nc" not in _CACHE:
        _CACHE["nc"] = build_nc()
    return _CACHE["nc"]


def make_in_maps(input, conv_w, conv_b):
    input = np.ascontiguousarray(np.asarray(input, dtype=np.float32))
    conv_w = np.asarray(conv_w, dtype=np.float32)
    conv_b = np.asarray(conv_b, dtype=np.float32)
    x = input.reshape(B, C, N)

    # conv weights: wmat[3s+i, o] = conv_w[o, i, dy+1, dx+1] for shift s
    wmat = np.empty((27, C), np.float32)
    for s, (dy, dx) in enumerate(SHIFTS):
        wmat[3 * s:3 * s + 3, :] = conv_w[:, :, dy + 1, dx + 1].T  # [i, o]
    wmat = wmat.astype(ml_dtypes.bfloat16)
    eye4x = np.zeros((128, 4), np.float32)
    for j in range(4):
        eye4x[32 * j:32 * j + 4, :] = np.eye(4, dtype=np.float32)
    id128 = np.eye(128, dtype=np.float32)

    in_maps = []
    for b in range(B):
        xb = x[b]
        xk_pad = np.zeros((128, N), np.float32)
        xk_pad[:C] = xb
        v4 = np.ones((128, KB, 4), np.float32)
        v4[:, :, :3] = xb.reshape(C, KB, 128).transpose(2, 1, 0)
        v4 = v4.reshape(128, KB * 4).astype(ml_dtypes.bfloat16)
        for j in range(BANDS):
            r0 = j * BAND_ROWS
            lo = r0 - 1
            xqb = np.zeros((C, HALO_ROWS, W), np.float32)
            mk = np.zeros((C, HALO_ROWS, W), np.float32)
            a = max(lo, 0)
            bb = min(lo + HALO_ROWS, H)
            xqb[:, a - lo:bb - lo, :] = input[b][:, a:bb, :]
            mk[:, a - lo:bb - lo, :] = 1.0
            xq_pad = np.zeros((128, QPAD), np.float32)
            xq_pad[:C, :QN] = xqb.reshape(C, QN)
            mk_pad = np.zeros((C, QPAD), np.float32)
            mk_pad[:, :QN] = mk.reshape(C, QN)
            residb = np.zeros((C, BAND_ROWS, WP), np.float32)
            residb[:, :, 1:1 + W] = (input[b][:, r0:r0 + BAND_ROWS, :]
                                     + conv_b[:, None, None])
            in_maps.append({
                "xk": xk_pad,
                "xq": xq_pad,
                "v4": v4,
                "mask": mk_pad,
                "resid": residb.reshape(C, CONVN),
                "wmat": wmat,
                "eye4x": eye4x,
                "id128": id128,
            })
    return in_maps


def run(input, conv_w, conv_b, trace=False, **spmd_kwargs):
    in_maps = make_in_maps(input, conv_w, conv_b)
    res = run_bass_kernel_spmd(_get_nc(), in_maps, list(range(2 * BANDS)),
                               trace=trace, **spmd_kwargs)
    out = np.empty((B, C, H, W), np.float32)
    for b in range(B):
        for j in range(BANDS):
            band = res.results[b * BANDS + j]["out"]
            out[b, :, j * BAND_ROWS:(j + 1) * BAND_ROWS, :] = (
                band.reshape(C, BAND_ROWS, W))
    return out, res


def kernel(input, conv_w, conv_b) -> np.ndarray:
    out, _ = run(input, conv_w, conv_b)
    return out


# revision 30
# speedup vs baseline: 1.0118x; 1.0118x over previous
"""Trainium2 Bass kernel for the non-local attention denoising block.

Computation (per batch b of the [2, 3, 96, 96] input):
    x      = input[b].reshape(3, 9216)                  # [C, N]
    S      = x^T x / sqrt(3)                            # [N, N] never materialized
    A      = softmax(S, axis=1)
    f      = (A @ x^T)^T                                # [C, N]
    out[b] = input[b] + conv3x3_same(f) + conv_b

Sharding: 8 cores = 2 batches x 4 query bands of 24 image rows. Each core
computes its band (+1 halo image row each side so the 3x3 conv needs no
cross-core exchange) against the full key sequence, flash-attention style:

  - mm1: S^T tile [128 k-part, 512 q-free] = matmul(lhsT=xk block, rhs=xq
    chunk) in float32r. The C=3 contraction is zero-padded to K=128 because
    the PE streams at half rate (427ns vs 216/313ns per 512 cols) whenever
    the moving operand has fewer than 128 partitions.
  - exp on the scalar engine, 3 k-blocks per instruction (free dim 1536)
    to amortize the ~352-cycle ACT instruction overhead; output bf16.
  - mm2: U^T[4, q] += matmul(lhsT=v4[kb] ([128, 4] = x^T with an appended
    ones column so row 3 accumulates the softmax denominator Z), rhs=e^T).
    The three k-blocks of a group go to three PE column groups (PSUM rows
    32j..32j+3) so their drains proceed concurrently (M=4 matmuls are
    drain-bound at 427ns otherwise).
  - softmax divide: PE-transpose the three U^T partials into one [128, 4]
    (transpose-and-sum via accumulating matmuls against a replicated I4),
    DVE reciprocal + per-partition tensor_scalar multiply, PE-transpose
    back. (Compute engines cannot address partition bases other than
    0 mod 32, so all cross-partition movement goes through the PE.)
  - conv: F goes into a width-98 zero-padded row layout (wrap columns and
    out-of-image rows read zeros = SAME padding), 9 shifted SBUF->SBUF DMA
    copies build a [27, 2352] stack, and the whole 3x3 conv is 5 K=27
    matmuls. Residual + bias are added in the padded layout; the final DMA
    strides the padding away.

No [N, N] tensor ever touches HBM; per-core HBM traffic is ~6 MB (dominated
by the zero-padded xk/xq loads, which overlap compute).
"""

import math
import os
import sys

for _p in (
    "/opt/trn_rl_repo",
    "/root/.axon_site",
    "/root/.axon_site/_ro/trn_rl_repo",
    "/root/.axon_site/_ro/pypackages",
):
    if os.path.isdir(_p) and _p not in sys.path:
        sys.path.append(_p)

import ml_dtypes  # noqa: E402
import numpy as np  # noqa: E402

import concourse.bacc as bacc  # noqa: E402
import concourse.bass as bass  # noqa: E402
import concourse.tile as tile  # noqa: E402
from concourse import mybir  # noqa: E402
from concourse.bass_utils import run_bass_kernel_spmd  # noqa: E402

# Problem shape (hardcoded per the harness contract).
B, C, H, W = 2, 3, 96, 96
N = H * W                      # 9216 spatial positions (keys)
BANDS = 4                      # query bands per batch
BAND_ROWS = H // BANDS         # 24 image rows per band
BAND_Q = BAND_ROWS * W         # 2304 output queries per core
HALO_ROWS = BAND_ROWS + 2      # +1 halo row each side for the conv
QN = HALO_ROWS * W             # 2496 computed queries per core
QPAD = QN                      # 2496 = 4 x 512 + 448 matmul chunks
KB = N // 128                  # 72 key blocks of 128
KB_GRP = 3                     # k-blocks fused per exp instruction
QCHUNK = 512
NQC = 5                        # 4 x 512 + 448
WP = W + 2                     # padded image row width for the conv
CONVN = BAND_ROWS * WP         # 2352 conv output positions (padded layout)
FPN = HALO_ROWS * WP + 4       # F_p length (+4 slack for shift reads)
INV_SQRT_C = 1.0 / math.sqrt(C)

F32 = mybir.dt.float32
F32R = mybir.dt.float32r
BF16 = mybir.dt.bfloat16
EXP = mybir.ActivationFunctionType.Exp

SHIFTS = [(dy, dx) for dy in (-1, 0, 1) for dx in (-1, 0, 1)]
MM1_FLAT = True            # K=128 zero-padded mm1 (K<128 slices miscompute)
TRS_DT_ENV = False         # f32 transposes (f32r transposes crash at runtime)


def build_nc() -> bass.Bass:
    nc = bacc.Bacc()

    xk = nc.declare_dram_parameter("xk", [128, N], F32R, isOutput=False)
    xq = nc.declare_dram_parameter("xq", [128, QPAD], F32R, isOutput=False)
    v4 = nc.declare_dram_parameter("v4", [128, KB * 4], BF16, isOutput=False)
    mask = nc.declare_dram_parameter("mask", [C, QPAD], F32, isOutput=False)
    resid = nc.declare_dram_parameter("resid", [C, CONVN], F32, isOutput=False)
    wmat = nc.declare_dram_parameter("wmat", [27, C], BF16, isOutput=False)
    TRS = F32R if TRS_DT_ENV else F32
    eye4x = nc.declare_dram_parameter("eye4x", [128, 4], TRS, isOutput=False)
    id128 = nc.declare_dram_parameter("id128", [128, 128], TRS, isOutput=False)
    out = nc.declare_dram_parameter("out", [C, BAND_Q], F32, isOutput=True)

    NXKC = 8                     # xk arrives in 8 chunk DMAs
    XKC = N // NXKC

    with tile.TileContext(nc) as tc, \
            tc.tile_pool(name="persist", bufs=1) as P, \
            tc.tile_pool(name="work", bufs=2) as WK, \
            tc.tile_pool(name="small", bufs=3) as SM, \
            tc.tile_pool(name="epool", bufs=3) as EPl, \
            tc.tile_pool(name="spool", bufs=2, space="PSUM") as SP, \
            tc.tile_pool(name="upool", bufs=1, space="PSUM") as UP, \
            tc.tile_pool(name="tpool", bufs=1, space="PSUM") as TP:

        # ---- load inputs -------------------------------------------------
        xq_sb = P.tile([128, QPAD], F32R, tag="xq", name="xq_sb")
        xk_sb = P.tile([128, N], F32R, tag="xk", name="xk_sb")
        v4_sb = P.tile([128, KB * 4], BF16, tag="v4", name="v4_sb")
        for c0 in range(0, QPAD, QCHUNK):
            cn_ = min(QCHUNK, QPAD - c0)
            nc.sync.dma_start(out=xq_sb[:, c0:c0 + cn_],
                              in_=xq[:, c0:c0 + cn_])
        # small first chunk so the first k-groups start sooner
        xk_edges = [0, 384, 1152] + [1152 + XKC * i for i in range(1, NXKC)] + [N]
        for a_, b_ in zip(xk_edges, xk_edges[1:]):
            if b_ > a_:
                nc.sync.dma_start(out=xk_sb[:, a_:b_], in_=xk[:, a_:b_])
        nc.sync.dma_start(out=v4_sb, in_=v4[:])
        mask_sb = P.tile([C, QPAD], F32, tag="mask", name="mask_sb")
        nc.sync.dma_start(out=mask_sb, in_=mask[:])
        resid_sb = P.tile([C, CONVN], F32, tag="resid", name="resid_sb")
        nc.sync.dma_start(out=resid_sb, in_=resid[:])
        wmat_sb = P.tile([27, C], BF16, tag="wmat", name="wmat_sb")
        nc.sync.dma_start(out=wmat_sb, in_=wmat[:])
        eye4x_sb = P.tile([128, 4], TRS, tag="eye4x", name="eye4x_sb")
        nc.sync.dma_start(out=eye4x_sb, in_=eye4x[:])
        id128_sb = P.tile([128, 128], TRS, tag="id128", name="id128_sb")
        nc.sync.dma_start(out=id128_sb, in_=id128[:])

        F_sb = P.tile([C, QPAD], F32, tag="F", name="F_sb")
        F_m = P.tile([C, QPAD], F32, tag="Fm", name="F_m")
        F_p = P.tile([C, FPN], BF16, tag="Fp", name="F_p")
        nc.vector.memset(F_p, 0.0)
        F_sh = P.tile([27, CONVN], BF16, tag="Fsh", name="F_sh")
        nc.vector.memset(F_sh, 0.0)
        out_pad = P.tile([C, CONVN], F32, tag="opad", name="out_pad")
        F_p_r = F_p[:, 0:HALO_ROWS * WP].rearrange("p (r w) -> p r w", w=WP)
        F_m_r = F_m[:, 0:QN].rearrange("p (r w) -> p r w", w=W)
        pstate = {"rows": 0, "cols": 0, "rc": 0}
        CONV_CH = 392

        def emit_pipeline(rows_now):
            if rows_now > pstate["rows"]:
                nc.vector.tensor_copy(
                    F_p_r[:, pstate["rows"]:rows_now, 1:1 + W],
                    F_m_r[:, pstate["rows"]:rows_now, :])
                pstate["rows"] = rows_now
            cols_now = max(0, min(CONVN, rows_now * WP - 197))
            if rows_now == HALO_ROWS:
                cols_now = CONVN
            if cols_now > pstate["cols"]:
                for s, (dy, dx) in enumerate(SHIFTS):
                    off = WP + dy * WP + dx
                    a = max(pstate["cols"], -off)
                    if cols_now > a:
                        nc.gpsimd.dma_start(
                            out=F_sh[3 * s:3 * s + 3, a:cols_now],
                            in_=F_p[:, a + off:cols_now + off])
                pstate["cols"] = cols_now
            while pstate["rc"] < CONVN // CONV_CH and                     (pstate["rc"] + 1) * CONV_CH <= pstate["cols"]:
                c0 = pstate["rc"] * CONV_CH
                cv_ps = TP.tile([C, CONV_CH], F32, tag="T", name="cv_ps",
                                padded_shape=[C, QCHUNK])
                nc.tensor.matmul(cv_ps, lhsT=wmat_sb,
                                 rhs=F_sh[:, c0:c0 + CONV_CH],
                                 start=True, stop=True)
                nc.vector.tensor_add(out_pad[:, c0:c0 + CONV_CH], cv_ps,
                                     resid_sb[:, c0:c0 + CONV_CH])
                pstate["rc"] += 1
        # ---- flash attention main loop ----------------------------------
        for qc in range(NQC):
            q0 = qc * QCHUNK
            qn = min(QCHUNK, QPAD - q0)
            U_ps = UP.tile([128, qn], F32, tag="U", name="U_ps",
                           padded_shape=[128, QCHUNK])
            for t in range(KB // KB_GRP):
                S_ps = SP.tile([128, KB_GRP, qn], F32, tag="S", name="S_ps",
                               padded_shape=[128, KB_GRP, QCHUNK])
                for j in range(KB_GRP):
                    kb = t * KB_GRP + j
                    nc.tensor.matmul(
                        S_ps[:, j, :],
                        lhsT=xk_sb[:, kb * 128:(kb + 1) * 128] if MM1_FLAT
                        else xk_sb[32 * j:32 * j + C, kb * 128:(kb + 1) * 128],
                        rhs=xq_sb[:, q0:q0 + qn] if MM1_FLAT
                        else xq_sb[32 * j:32 * j + C, q0:q0 + qn],
                        start=True, stop=True,
                    )
                e_sb = EPl.tile([128, KB_GRP, qn], BF16, tag="e", name="e_sb",
                                padded_shape=[128, KB_GRP, QCHUNK])
                nc.scalar.activation(e_sb, S_ps, EXP, scale=INV_SQRT_C)
                for j in range(KB_GRP):
                    kb = t * KB_GRP + j
                    nc.tensor.matmul(
                        U_ps[32 * j:32 * j + 4, 0:qn],
                        lhsT=v4_sb[:, kb * 4:kb * 4 + 4],
                        rhs=e_sb[:, j, :],
                        start=(t == 0), stop=(t == KB // KB_GRP - 1),
                        skip_group_check=True,
                    )

            # softmax division via PE transposes (partition-aligned only)
            U_sb = WK.tile([128, qn], TRS, tag="Usb", name="U_sb",
                           padded_shape=[128, QCHUNK])
            nc.vector.tensor_copy(U_sb, U_ps)
            for c in range((qn + 127) // 128):
                cw = min(128, qn - c * 128)
                # last qc: main loop is done, the S slots are free -- use
                # them so the exposed tail divide double-buffers and the
                # TP slot stays free for the conv chunks
                TPt = (SP if qc == NQC - 1 else TP).tile(
                    [128, 132], F32,
                    tag="S" if qc == NQC - 1 else "T", name="TPt",
                    padded_shape=None if qc == NQC - 1 else [128, QCHUNK])
                Ut = TPt[0:cw, 0:4]
                # transpose-and-sum the three column-group partials
                for j in range(KB_GRP):
                    nc.tensor.matmul(
                        Ut,
                        lhsT=U_sb[32 * j:32 * j + 4, c * 128:c * 128 + cw],
                        rhs=eye4x_sb[32 * j:32 * j + 4, :],
                        start=(j == 0), stop=(j == KB_GRP - 1),
                        skip_group_check=True,
                    )
                r_sb = SM.tile([128, 1], F32, tag="r", name="r_sb")
                nc.vector.reciprocal(r_sb[0:cw, :], Ut[:, 3:4])
                Ft_sb = SM.tile([128, 4], TRS, tag="Ft", name="Ft_sb")
                nc.vector.tensor_scalar_mul(Ft_sb[0:cw, :], Ut, r_sb[0:cw, :])
                Fb = TPt[0:4, 4:4 + cw]
                nc.tensor.matmul(Fb, lhsT=Ft_sb[0:cw, :],
                                 rhs=id128_sb[0:cw, 0:cw],
                                 start=True, stop=True)
                nc.vector.tensor_copy(F_sb[:, q0 + c * 128:q0 + c * 128 + cw],
                                      Fb[0:3, :])
            nc.vector.tensor_mul(F_m[:, q0:q0 + qn],
                                 F_sb[:, q0:q0 + qn],
                                 mask_sb[:, q0:q0 + qn])
            if qc >= 2:
                emit_pipeline(min(HALO_ROWS, (q0 + qn) // W))


        # ---- tail: (conv emitted per-qc above) ---------------------------
        out_pad_r = out_pad.rearrange("p (r w) -> p r w", w=WP)
        nc.sync.dma_start(out=out[:].rearrange("p (r w) -> p r w", w=W),
                          in_=out_pad_r[:, :, 1:1 + W])

    nc.compile()
    return nc


_CACHE: dict = {}


def _get_nc() -> bass.Bass:
    if "nc" not in _CACHE:
        _CACHE["nc"] = build_nc()
    return _CACHE["nc"]


def make_in_maps(input, conv_w, conv_b):
    input = np.ascontiguousarray(np.asarray(input, dtype=np.float32))
    conv_w = np.asarray(conv_w, dtype=np.float32)
    conv_b = np.asarray(conv_b, dtype=np.float32)
    x = input.reshape(B, C, N)

    # conv weights: wmat[3s+i, o] = conv_w[o, i, dy+1, dx+1] for shift s
    wmat = np.empty((27, C), np.float32)
    for s, (dy, dx) in enumerate(SHIFTS):
        wmat[3 * s:3 * s + 3, :] = conv_w[:, :, dy + 1, dx + 1].T  # [i, o]
    wmat = wmat.astype(ml_dtypes.bfloat16)
    eye4x = np.zeros((128, 4), np.float32)
    for j in range(4):
        eye4x[32 * j:32 * j + 4, :] = np.eye(4, dtype=np.float32)
    id128 = np.eye(128, dtype=np.float32)

    in_maps = []
    for b in range(B):
        xb = x[b]
        xk_pad = np.zeros((128, N), np.float32)
        xk_pad[:C] = xb
        v4 = np.ones((128, KB, 4), np.float32)
        v4[:, :, :3] = xb.reshape(C, KB, 128).transpose(2, 1, 0)
        v4 = v4.reshape(128, KB * 4).astype(ml_dtypes.bfloat16)
        for j in range(BANDS):
            r0 = j * BAND_ROWS
            lo = r0 - 1
            xqb = np.zeros((C, HALO_ROWS, W), np.float32)
            mk = np.zeros((C, HALO_ROWS, W), np.float32)
            a = max(lo, 0)
            bb = min(lo + HALO_ROWS, H)
            xqb[:, a - lo:bb - lo, :] = input[b][:, a:bb, :]
            mk[:, a - lo:bb - lo, :] = 1.0
            xq_pad = np.zeros((128, QPAD), np.float32)
            xq_pad[:C, :QN] = xqb.reshape(C, QN)
            mk_pad = np.zeros((C, QPAD), np.float32)
            mk_pad[:, :QN] = mk.reshape(C, QN)
            residb = np.zeros((C, BAND_ROWS, WP), np.float32)
            residb[:, :, 1:1 + W] = (input[b][:, r0:r0 + BAND_ROWS, :]
                                     + conv_b[:, None, None])
            in_maps.append({
                "xk": xk_pad,
                "xq": xq_pad,
                "v4": v4,
                "mask": mk_pad,
                "resid": residb.reshape(C, CONVN),
                "wmat": wmat,
                "eye4x": eye4x,
                "id128": id128,
            })
    return in_maps


def run(input, conv_w, conv_b, trace=False, **spmd_kwargs):
    in_maps = make_in_maps(input, conv_w, conv_b)
    res = run_bass_kernel_spmd(_get_nc(), in_maps, list(range(2 * BANDS)),
                               trace=trace, **spmd_kwargs)
    out = np.empty((B, C, H, W), np.float32)
    for b in range(B):
        for j in range(BANDS):
            band = res.results[b * BANDS + j]["out"]
            out[b, :, j * BAND_ROWS:(j + 1) * BAND_ROWS, :] = (
                band.reshape(C, BAND_ROWS, W))
    return out, res


def kernel(input, conv_w, conv_b) -> np.ndarray:
    out, _ = run(input, conv_w, conv_b)
    return out


# revision 32
# speedup vs baseline: 1.0218x; 1.0098x over previous
"""Trainium2 Bass kernel for the non-local attention denoising block.

Computation (per batch b of the [2, 3, 96, 96] input):
    x      = input[b].reshape(3, 9216)                  # [C, N]
    S      = x^T x / sqrt(3)                            # [N, N] never materialized
    A      = softmax(S, axis=1)
    f      = (A @ x^T)^T                                # [C, N]
    out[b] = input[b] + conv3x3_same(f) + conv_b

Sharding: 8 cores = 2 batches x 4 query bands of 24 image rows. Each core
computes its band (+1 halo image row each side so the 3x3 conv needs no
cross-core exchange) against the full key sequence, flash-attention style:

  - mm1: S^T tile [128 k-part, 512 q-free] = matmul(lhsT=xk block, rhs=xq
    chunk) in float32r. The C=3 contraction is zero-padded to K=128 because
    the PE streams at half rate (427ns vs 216/313ns per 512 cols) whenever
    the moving operand has fewer than 128 partitions.
  - exp on the scalar engine, 3 k-blocks per instruction (free dim 1536)
    to amortize the ~352-cycle ACT instruction overhead; output bf16.
  - mm2: U^T[4, q] += matmul(lhsT=v4[kb] ([128, 4] = x^T with an appended
    ones column so row 3 accumulates the softmax denominator Z), rhs=e^T).
    The three k-blocks of a group go to three PE column groups (PSUM rows
    32j..32j+3) so their drains proceed concurrently (M=4 matmuls are
    drain-bound at 427ns otherwise).
  - softmax divide: PE-transpose the three U^T partials into one [128, 4]
    (transpose-and-sum via accumulating matmuls against a replicated I4),
    DVE reciprocal + per-partition tensor_scalar multiply, PE-transpose
    back. (Compute engines cannot address partition bases other than
    0 mod 32, so all cross-partition movement goes through the PE.)
  - conv: F goes into a width-98 zero-padded row layout (wrap columns and
    out-of-image rows read zeros = SAME padding), 9 shifted SBUF->SBUF
    gpsimd DMA copies build a [27, 2352] stack, and the whole 3x3 conv is
    six K=27 matmuls. The conv is pipelined into the flash loop: as soon as
    a query chunk's F rows are final (qc>=2), the corresponding F_p rows,
    F_sh columns and conv chunks are emitted, so only the last chunk's
    chain remains in the tail. The last chunk's divide epilogue allocates
    its transpose PSUM from the then-free S slots to double-buffer the
    exposed tail. Residual + bias are added in the padded layout; the
    final DMA strides the padding away.

No [N, N] tensor ever touches HBM; per-core HBM traffic is ~6 MB (dominated
by the zero-padded xk/xq loads, which overlap compute).
"""

import math
import os
import sys

for _p in (
    "/opt/trn_rl_repo",
    "/root/.axon_site",
    "/root/.axon_site/_ro/trn_rl_repo",
    "/root/.axon_site/_ro/pypackages",
):
    if os.path.isdir(_p) and _p not in sys.path:
        sys.path.append(_p)

import ml_dtypes  # noqa: E402
import numpy as np  # noqa: E402

import concourse.bacc as bacc  # noqa: E402
import concourse.bass as bass  # noqa: E402
import concourse.tile as tile  # noqa: E402
from concourse import mybir  # noqa: E402
from concourse.bass_utils import run_bass_kernel_spmd  # noqa: E402

# Problem shape (hardcoded per the harness contract).
B, C, H, W = 2, 3, 96, 96
N = H * W                      # 9216 spatial positions (keys)
BANDS = 4                      # query bands per batch
BAND_ROWS = H // BANDS         # 24 image rows per band
BAND_Q = BAND_ROWS * W         # 2304 output queries per core
HALO_ROWS = BAND_ROWS + 2      # +1 halo row each side for the conv
QN = HALO_ROWS * W             # 2496 computed queries per core
QPAD = QN                      # 2496 = 4 x 512 + 448 matmul chunks
KB = N // 128                  # 72 key blocks of 128
KB_GRP = 3                     # k-blocks fused per exp instruction
QCHUNK = 512
NQC = 5                        # 4 x 512 + 448
WP = W + 2                     # padded image row width for the conv
CONVN = BAND_ROWS * WP         # 2352 conv output positions (padded layout)
FPN = HALO_ROWS * WP + 4       # F_p length (+4 slack for shift reads)
INV_SQRT_C = 1.0 / math.sqrt(C)

F32 = mybir.dt.float32
F32R = mybir.dt.float32r
BF16 = mybir.dt.bfloat16
EXP = mybir.ActivationFunctionType.Exp

SHIFTS = [(dy, dx) for dy in (-1, 0, 1) for dx in (-1, 0, 1)]
MM1_FLAT = True            # K=128 zero-padded mm1 (K<128 slices miscompute)
TRS_DT_ENV = False         # f32 transposes (f32r transposes crash at runtime)


def build_nc() -> bass.Bass:
    nc = bacc.Bacc()

    xk = nc.declare_dram_parameter("xk", [128, N], F32R, isOutput=False)
    xq = nc.declare_dram_parameter("xq", [128, QPAD], F32R, isOutput=False)
    v4 = nc.declare_dram_parameter("v4", [128, KB * 4], BF16, isOutput=False)
    mask = nc.declare_dram_parameter("mask", [C, QPAD], F32, isOutput=False)
    resid = nc.declare_dram_parameter("resid", [C, CONVN], F32, isOutput=False)
    wmat = nc.declare_dram_parameter("wmat", [27, C], BF16, isOutput=False)
    TRS = F32R if TRS_DT_ENV else F32
    eye4x = nc.declare_dram_parameter("eye4x", [128, 4], TRS, isOutput=False)
    id128 = nc.declare_dram_parameter("id128", [128, 128], TRS, isOutput=False)
    out = nc.declare_dram_parameter("out", [C, BAND_Q], F32, isOutput=True)

    NXKC = 8                     # xk arrives in 8 chunk DMAs
    XKC = N // NXKC

    with tile.TileContext(nc) as tc, \
            tc.tile_pool(name="persist", bufs=1) as P, \
            tc.tile_pool(name="work", bufs=2) as WK, \
            tc.tile_pool(name="small", bufs=3) as SM, \
            tc.tile_pool(name="epool", bufs=4) as EPl, \
            tc.tile_pool(name="spool", bufs=2, space="PSUM") as SP, \
            tc.tile_pool(name="upool", bufs=1, space="PSUM") as UP, \
            tc.tile_pool(name="tpool", bufs=1, space="PSUM") as TP:

        # ---- load inputs -------------------------------------------------
        xq_sb = P.tile([128, QPAD], F32R, tag="xq", name="xq_sb")
        xk_sb = P.tile([128, N], F32R, tag="xk", name="xk_sb")
        v4_sb = P.tile([128, KB * 4], BF16, tag="v4", name="v4_sb")
        for c0 in range(0, QPAD, QCHUNK):
            cn_ = min(QCHUNK, QPAD - c0)
            nc.sync.dma_start(out=xq_sb[:, c0:c0 + cn_],
                              in_=xq[:, c0:c0 + cn_])
        # small first chunk so the first k-groups start sooner
        xk_edges = [0, 384, 1152] + [1152 + XKC * i for i in range(1, NXKC)] + [N]
        for a_, b_ in zip(xk_edges, xk_edges[1:]):
            if b_ > a_:
                nc.sync.dma_start(out=xk_sb[:, a_:b_], in_=xk[:, a_:b_])
        nc.sync.dma_start(out=v4_sb, in_=v4[:])
        mask_sb = P.tile([C, QPAD], F32, tag="mask", name="mask_sb")
        nc.sync.dma_start(out=mask_sb, in_=mask[:])
        resid_sb = P.tile([C, CONVN], F32, tag="resid", name="resid_sb")
        nc.sync.dma_start(out=resid_sb, in_=resid[:])
        wmat_sb = P.tile([27, C], BF16, tag="wmat", name="wmat_sb")
        nc.sync.dma_start(out=wmat_sb, in_=wmat[:])
        eye4x_sb = P.tile([128, 4], TRS, tag="eye4x", name="eye4x_sb")
        nc.sync.dma_start(out=eye4x_sb, in_=eye4x[:])
        id128_sb = P.tile([128, 128], TRS, tag="id128", name="id128_sb")
        nc.sync.dma_start(out=id128_sb, in_=id128[:])

        F_sb = P.tile([C, QPAD], F32, tag="F", name="F_sb")
        F_m = P.tile([C, QPAD], F32, tag="Fm", name="F_m")
        F_p = P.tile([C, FPN], BF16, tag="Fp", name="F_p")
        nc.vector.memset(F_p, 0.0)
        F_sh = P.tile([27, CONVN], BF16, tag="Fsh", name="F_sh")
        nc.vector.memset(F_sh, 0.0)
        out_pad = P.tile([C, CONVN], F32, tag="opad", name="out_pad")
        F_p_r = F_p[:, 0:HALO_ROWS * WP].rearrange("p (r w) -> p r w", w=WP)
        F_m_r = F_m[:, 0:QN].rearrange("p (r w) -> p r w", w=W)
        pstate = {"rows": 0, "cols": 0, "rc": 0}
        CONV_CH = 392

        def emit_pipeline(rows_now):
            if rows_now > pstate["rows"]:
                nc.vector.tensor_copy(
                    F_p_r[:, pstate["rows"]:rows_now, 1:1 + W],
                    F_m_r[:, pstate["rows"]:rows_now, :])
                pstate["rows"] = rows_now
            cols_now = max(0, min(CONVN, rows_now * WP - 197))
            if rows_now == HALO_ROWS:
                cols_now = CONVN
            if cols_now > pstate["cols"]:
                final = rows_now == HALO_ROWS
                for s, (dy, dx) in enumerate(SHIFTS):
                    off = WP + dy * WP + dx
                    a = max(pstate["cols"], -off)
                    if cols_now > a:
                        # on the final emission the sync queue is idle;
                        # alternating engines halves the serial issue time
                        eng = nc.sync if (final and s % 2 == 0) else nc.gpsimd
                        eng.dma_start(
                            out=F_sh[3 * s:3 * s + 3, a:cols_now],
                            in_=F_p[:, a + off:cols_now + off])
                pstate["cols"] = cols_now
            while pstate["rc"] < CONVN // CONV_CH and                     (pstate["rc"] + 1) * CONV_CH <= pstate["cols"]:
                c0 = pstate["rc"] * CONV_CH
                cv_ps = TP.tile([C, CONV_CH], F32, tag="T", name="cv_ps",
                                padded_shape=[C, QCHUNK])
                nc.tensor.matmul(cv_ps, lhsT=wmat_sb,
                                 rhs=F_sh[:, c0:c0 + CONV_CH],
                                 start=True, stop=True)
                nc.vector.tensor_add(out_pad[:, c0:c0 + CONV_CH], cv_ps,
                                     resid_sb[:, c0:c0 + CONV_CH])
                pstate["rc"] += 1
        # ---- flash attention main loop ----------------------------------
        for qc in range(NQC):
            q0 = qc * QCHUNK
            qn = min(QCHUNK, QPAD - q0)
            U_ps = UP.tile([128, qn], F32, tag="U", name="U_ps",
                           padded_shape=[128, QCHUNK])
            for t in range(KB // KB_GRP):
                S_ps = SP.tile([128, KB_GRP, qn], F32, tag="S", name="S_ps",
                               padded_shape=[128, KB_GRP, QCHUNK])
                for j in range(KB_GRP):
                    kb = t * KB_GRP + j
                    nc.tensor.matmul(
                        S_ps[:, j, :],
                        lhsT=xk_sb[:, kb * 128:(kb + 1) * 128] if MM1_FLAT
                        else xk_sb[32 * j:32 * j + C, kb * 128:(kb + 1) * 128],
                        rhs=xq_sb[:, q0:q0 + qn] if MM1_FLAT
                        else xq_sb[32 * j:32 * j + C, q0:q0 + qn],
                        start=True, stop=True,
                    )
                e_sb = EPl.tile([128, KB_GRP, qn], BF16, tag="e", name="e_sb",
                                padded_shape=[128, KB_GRP, QCHUNK])
                nc.scalar.activation(e_sb, S_ps, EXP, scale=INV_SQRT_C)
                for j in range(KB_GRP):
                    kb = t * KB_GRP + j
                    nc.tensor.matmul(
                        U_ps[32 * j:32 * j + 4, 0:qn],
                        lhsT=v4_sb[:, kb * 4:kb * 4 + 4],
                        rhs=e_sb[:, j, :],
                        start=(t == 0), stop=(t == KB // KB_GRP - 1),
                        skip_group_check=True,
                    )

            # softmax division via PE transposes (partition-aligned only)
            U_sb = WK.tile([128, qn], TRS, tag="Usb", name="U_sb",
                           padded_shape=[128, QCHUNK])
            nc.vector.tensor_copy(U_sb, U_ps)
            for c in range((qn + 127) // 128):
                cw = min(128, qn - c * 128)
                # last qc: main loop is done, the S slots are free -- use
                # them so the exposed tail divide double-buffers and the
                # TP slot stays free for the conv chunks
                TPt = (SP if qc == NQC - 1 else TP).tile(
                    [128, 132], F32,
                    tag="S" if qc == NQC - 1 else "T", name="TPt",
                    padded_shape=None if qc == NQC - 1 else [128, QCHUNK])
                Ut = TPt[0:cw, 0:4]
                # transpose-and-sum the three column-group partials
                for j in range(KB_GRP):
                    nc.tensor.matmul(
                        Ut,
                        lhsT=U_sb[32 * j:32 * j + 4, c * 128:c * 128 + cw],
                        rhs=eye4x_sb[32 * j:32 * j + 4, :],
                        start=(j == 0), stop=(j == KB_GRP - 1),
                        skip_group_check=True,
                    )
                r_sb = SM.tile([128, 1], F32, tag="r", name="r_sb")
                nc.vector.reciprocal(r_sb[0:cw, :], Ut[:, 3:4])
                Ft_sb = SM.tile([128, 4], TRS, tag="Ft", name="Ft_sb")
                nc.vector.tensor_scalar_mul(Ft_sb[0:cw, :], Ut, r_sb[0:cw, :])
                Fb = TPt[0:4, 4:4 + cw]
                nc.tensor.matmul(Fb, lhsT=Ft_sb[0:cw, :],
                                 rhs=id128_sb[0:cw, 0:cw],
                                 start=True, stop=True)
                nc.vector.tensor_copy(F_sb[:, q0 + c * 128:q0 + c * 128 + cw],
                                      Fb[0:3, :])
            nc.vector.tensor_mul(F_m[:, q0:q0 + qn],
                                 F_sb[:, q0:q0 + qn],
                                 mask_sb[:, q0:q0 + qn])
            if qc >= 2:
                emit_pipeline(min(HALO_ROWS, (q0 + qn) // W))


        # ---- tail: (conv emitted per-qc above) ---------------------------
        out_pad_r = out_pad.rearrange("p (r w) -> p r w", w=WP)
        nc.sync.dma_start(out=out[:].rearrange("p (r w) -> p r w", w=W),
                          in_=out_pad_r[:, :, 1:1 + W])

    nc.compile()
    return nc


_CACHE: dict = {}


def _get_nc() -> bass.Bass:
    if "nc" not in _CACHE:
        _CACHE["nc"] = build_nc()
    return _CACHE["nc"]


def make_in_maps(input, conv_w, conv_b):
    input = np.ascontiguousarray(np.asarray(input, dtype=np.float32))
    conv_w = np.asarray(conv_w, dtype=np.float32)
    conv_b = np.asarray(conv_b, dtype=np.float32)
    x = input.reshape(B, C, N)

    # conv weights: wmat[3s+i, o] = conv_w[o, i, dy+1, dx+1] for shift s
    wmat = np.empty((27, C), np.float32)
    for s, (dy, dx) in enumerate(SHIFTS):
        wmat[3 * s:3 * s + 3, :] = conv_w[:, :, dy + 1, dx + 1].T  # [i, o]
    wmat = wmat.astype(ml_dtypes.bfloat16)
    eye4x = np.zeros((128, 4), np.float32)
    for j in range(4):
        eye4x[32 * j:32 * j + 4, :] = np.eye(4, dtype=np.float32)
    id128 = np.eye(128, dtype=np.float32)

    in_maps = []
    for b in range(B):
        xb = x[b]
        xk_pad = np.zeros((128, N), np.float32)
        xk_pad[:C] = xb
        v4 = np.ones((128, KB, 4), np.float32)
        v4[:, :, :3] = xb.reshape(C, KB, 128).transpose(2, 1, 0)
        v4 = v4.reshape(128, KB * 4).astype(ml_dtypes.bfloat16)
        for j in range(BANDS):
            r0 = j * BAND_ROWS
            lo = r0 - 1
            xqb = np.zeros((C, HALO_ROWS, W), np.float32)
            mk = np.zeros((C, HALO_ROWS, W), np.float32)
            a = max(lo, 0)
            bb = min(lo + HALO_ROWS, H)
            xqb[:, a - lo:bb - lo, :] = input[b][:, a:bb, :]
            mk[:, a - lo:bb - lo, :] = 1.0
            xq_pad = np.zeros((128, QPAD), np.float32)
            xq_pad[:C, :QN] = xqb.reshape(C, QN)
            mk_pad = np.zeros((C, QPAD), np.float32)
            mk_pad[:, :QN] = mk.reshape(C, QN)
            residb = np.zeros((C, BAND_ROWS, WP), np.float32)
            residb[:, :, 1:1 + W] = (input[b][:, r0:r0 + BAND_ROWS, :]
                                     + conv_b[:, None, None])
            in_maps.append({
                "xk": xk_pad,
                "xq": xq_pad,
                "v4": v4,
                "mask": mk_pad,
                "resid": residb.reshape(C, CONVN),
                "wmat": wmat,
                "eye4x": eye4x,
                "id128": id128,
            })
    return in_maps


def run(input, conv_w, conv_b, trace=False, **spmd_kwargs):
    in_maps = make_in_maps(input, conv_w, conv_b)
    res = run_bass_kernel_spmd(_get_nc(), in_maps, list(range(2 * BANDS)),
                               trace=trace, **spmd_kwargs)
    out = np.empty((B, C, H, W), np.float32)
    for b in range(B):
        for j in range(BANDS):
            band = res.results[b * BANDS + j]["out"]
            out[b, :, j * BAND_ROWS:(j + 1) * BAND_ROWS, :] = (
                band.reshape(C, BAND_ROWS, W))
    return out, res


def kernel(input, conv_w, conv_b) -> np.ndarray:
    out, _ = run(input, conv_w, conv_b)
    return out


# revision 35
# speedup vs baseline: 1.0620x; 1.0393x over previous
"""Trainium2 Bass kernel for the non-local attention denoising block.

Computation (per batch b of the [2, 3, 96, 96] input):
    x      = input[b].reshape(3, 9216)                  # [C, N]
    S      = x^T x / sqrt(3)                            # [N, N] never materialized
    A      = softmax(S, axis=1)
    f      = (A @ x^T)^T                                # [C, N]
    out[b] = input[b] + conv3x3_same(f) + conv_b

Sharding: 8 cores = 2 batches x 4 query bands of 24 image rows. Each core
computes its band (+1 halo image row each side so the 3x3 conv needs no
cross-core exchange) against the full key sequence, flash-attention style:

  - mm1: S^T tile [128 k-part, 512 q-free] = matmul(lhsT=xk block, rhs=xq
    chunk) in float32r. The C=3 contraction is zero-padded to K=128 because
    the PE streams at half rate (427ns vs 216/313ns per 512 cols) whenever
    the moving operand has fewer than 128 partitions.
  - exp on the scalar engine, 3 k-blocks per instruction (free dim 1536)
    to amortize the ~352-cycle ACT instruction overhead; output bf16.
  - mm2: U^T[4, q] += matmul(lhsT=v4[kb] ([128, 4] = x^T with an appended
    ones column so row 3 accumulates the softmax denominator Z), rhs=e^T).
    The three k-blocks of a group go to three PE column groups (PSUM rows
    32j..32j+3) so their drains proceed concurrently (M=4 matmuls are
    drain-bound at 427ns otherwise).
  - softmax divide: PE-transpose the three U^T partials into one [128, 4]
    (transpose-and-sum via accumulating matmuls against a replicated I4),
    DVE reciprocal + per-partition tensor_scalar multiply, PE-transpose
    back. (Compute engines cannot address partition bases other than
    0 mod 32, so all cross-partition movement goes through the PE.)
  - conv: F goes into a width-98 zero-padded row layout (wrap columns and
    out-of-image rows read zeros = SAME padding), 9 shifted SBUF->SBUF
    gpsimd DMA copies build a [27, 2352] stack, and the whole 3x3 conv is
    six K=27 matmuls. The conv is pipelined into the flash loop: as soon as
    a query chunk's F rows are final (qc>=2), the corresponding F_p rows,
    F_sh columns and conv chunks are emitted, so only the last chunk's
    chain remains in the tail. The last chunk's divide epilogue allocates
    its transpose PSUM from the then-free S slots to double-buffer the
    exposed tail. Residual + bias are added in the padded layout; the
    final DMA strides the padding away.

No [N, N] tensor ever touches HBM; per-core HBM traffic is ~6 MB (dominated
by the zero-padded xk/xq loads, which overlap compute).
"""

import math
import os
import sys

for _p in (
    "/opt/trn_rl_repo",
    "/root/.axon_site",
    "/root/.axon_site/_ro/trn_rl_repo",
    "/root/.axon_site/_ro/pypackages",
):
    if os.path.isdir(_p) and _p not in sys.path:
        sys.path.append(_p)

import ml_dtypes  # noqa: E402
import numpy as np  # noqa: E402

import concourse.bacc as bacc  # noqa: E402
import concourse.bass as bass  # noqa: E402
import concourse.tile as tile  # noqa: E402
from concourse import mybir  # noqa: E402
from concourse.bass_utils import run_bass_kernel_spmd  # noqa: E402

# Problem shape (hardcoded per the harness contract).
B, C, H, W = 2, 3, 96, 96
N = H * W                      # 9216 spatial positions (keys)
BANDS = 4                      # query bands per batch
BAND_ROWS = H // BANDS         # 24 image rows per band
BAND_Q = BAND_ROWS * W         # 2304 output queries per core
HALO_ROWS = BAND_ROWS + 2      # +1 halo row each side for the conv
QN = HALO_ROWS * W             # 2496 computed queries per core
QPAD = QN                      # 2496 = 4 x 512 + 448 matmul chunks
KB = N // 128                  # 72 key blocks of 128
KB_GRP = 3                     # k-blocks fused per exp instruction
QCHUNK = 512
NQC = 5                        # 4 x 512 + 448
WP = W + 2                     # padded image row width for the conv
CONVN = BAND_ROWS * WP         # 2352 conv output positions (padded layout)
FPN = HALO_ROWS * WP + 4       # F_p length (+4 slack for shift reads)
INV_SQRT_C = 1.0 / math.sqrt(C)

F32 = mybir.dt.float32
F32R = mybir.dt.float32r
BF16 = mybir.dt.bfloat16
EXP = mybir.ActivationFunctionType.Exp

SHIFTS = [(dy, dx) for dy in (-1, 0, 1) for dx in (-1, 0, 1)]
MM1_FLAT = True            # K=128 zero-padded mm1 (K<128 slices miscompute)
TRS_DT_ENV = False         # f32 transposes (f32r transposes crash at runtime)


def build_nc() -> bass.Bass:
    nc = bacc.Bacc()

    xk = nc.declare_dram_parameter("xk", [128, N], F32R, isOutput=False)
    xq = nc.declare_dram_parameter("xq", [128, QPAD], F32R, isOutput=False)
    v4 = nc.declare_dram_parameter("v4", [128, KB * 4], BF16, isOutput=False)
    mask = nc.declare_dram_parameter("mask", [C, QPAD], F32, isOutput=False)
    resid = nc.declare_dram_parameter("resid", [C, CONVN], F32, isOutput=False)
    wmat = nc.declare_dram_parameter("wmat", [27, C], BF16, isOutput=False)
    TRS = F32R if TRS_DT_ENV else F32
    eye4x = nc.declare_dram_parameter("eye4x", [128, 4], TRS, isOutput=False)
    id128 = nc.declare_dram_parameter("id128", [128, 128], TRS, isOutput=False)
    out = nc.declare_dram_parameter("out", [C, BAND_Q], F32, isOutput=True)

    NXKC = 8                     # xk arrives in 8 chunk DMAs
    XKC = N // NXKC

    with tile.TileContext(nc) as tc, \
            tc.tile_pool(name="persist", bufs=1) as P, \
            tc.tile_pool(name="work", bufs=2) as WK, \
            tc.tile_pool(name="small", bufs=3) as SM, \
            tc.tile_pool(name="epool", bufs=8) as EPl, \
            tc.tile_pool(name="spool", bufs=2, space="PSUM") as SP, \
            tc.tile_pool(name="upool", bufs=1, space="PSUM") as UP, \
            tc.tile_pool(name="tpool", bufs=1, space="PSUM") as TP:

        # ---- load inputs -------------------------------------------------
        xq_sb = P.tile([128, QPAD], F32R, tag="xq", name="xq_sb")
        xk_sb = P.tile([128, N], F32R, tag="xk", name="xk_sb")
        v4_sb = P.tile([128, KB * 4], BF16, tag="v4", name="v4_sb")
        for c0 in range(0, QPAD, QCHUNK):
            cn_ = min(QCHUNK, QPAD - c0)
            nc.sync.dma_start(out=xq_sb[:, c0:c0 + cn_],
                              in_=xq[:, c0:c0 + cn_])
        # small first chunk so the first k-groups start sooner
        xk_edges = [0, 384, 1152] + [1152 + XKC * i for i in range(1, NXKC)] + [N]
        for a_, b_ in zip(xk_edges, xk_edges[1:]):
            if b_ > a_:
                nc.sync.dma_start(out=xk_sb[:, a_:b_], in_=xk[:, a_:b_])
        nc.sync.dma_start(out=v4_sb, in_=v4[:])
        mask_sb = P.tile([C, QPAD], F32, tag="mask", name="mask_sb")
        nc.sync.dma_start(out=mask_sb, in_=mask[:])
        resid_sb = P.tile([C, CONVN], F32, tag="resid", name="resid_sb")
        nc.sync.dma_start(out=resid_sb, in_=resid[:])
        wmat_sb = P.tile([27, C], BF16, tag="wmat", name="wmat_sb")
        nc.sync.dma_start(out=wmat_sb, in_=wmat[:])
        eye4x_sb = P.tile([128, 4], TRS, tag="eye4x", name="eye4x_sb")
        nc.sync.dma_start(out=eye4x_sb, in_=eye4x[:])
        id128_sb = P.tile([128, 128], TRS, tag="id128", name="id128_sb")
        nc.sync.dma_start(out=id128_sb, in_=id128[:])

        F_sb = P.tile([C, QPAD], F32, tag="F", name="F_sb")
        F_m = P.tile([C, QPAD], F32, tag="Fm", name="F_m")
        F_p = P.tile([C, FPN], BF16, tag="Fp", name="F_p")
        nc.vector.memset(F_p, 0.0)
        F_sh = P.tile([27, CONVN], BF16, tag="Fsh", name="F_sh")
        nc.vector.memset(F_sh, 0.0)
        out_pad = P.tile([C, CONVN], F32, tag="opad", name="out_pad")
        F_p_r = F_p[:, 0:HALO_ROWS * WP].rearrange("p (r w) -> p r w", w=WP)
        F_m_r = F_m[:, 0:QN].rearrange("p (r w) -> p r w", w=W)
        pstate = {"rows": 0, "cols": 0, "rc": 0}
        CONV_CH = 392

        def emit_pipeline(rows_now):
            if rows_now > pstate["rows"]:
                nc.vector.tensor_copy(
                    F_p_r[:, pstate["rows"]:rows_now, 1:1 + W],
                    F_m_r[:, pstate["rows"]:rows_now, :])
                pstate["rows"] = rows_now
            cols_now = max(0, min(CONVN, rows_now * WP - 197))
            if rows_now == HALO_ROWS:
                cols_now = CONVN
            if cols_now > pstate["cols"]:
                final = rows_now == HALO_ROWS
                for s, (dy, dx) in enumerate(SHIFTS):
                    off = WP + dy * WP + dx
                    a = max(pstate["cols"], -off)
                    if cols_now > a:
                        # on the final emission the sync queue is idle;
                        # alternating engines halves the serial issue time
                        eng = nc.sync if (final and s % 2 == 0) else nc.gpsimd
                        eng.dma_start(
                            out=F_sh[3 * s:3 * s + 3, a:cols_now],
                            in_=F_p[:, a + off:cols_now + off])
                pstate["cols"] = cols_now
            while pstate["rc"] < CONVN // CONV_CH and                     (pstate["rc"] + 1) * CONV_CH <= pstate["cols"]:
                c0 = pstate["rc"] * CONV_CH
                cv_ps = TP.tile([C, CONV_CH], F32, tag="T", name="cv_ps",
                                padded_shape=[C, QCHUNK])
                nc.tensor.matmul(cv_ps, lhsT=wmat_sb,
                                 rhs=F_sh[:, c0:c0 + CONV_CH],
                                 start=True, stop=True)
                nc.vector.tensor_add(out_pad[:, c0:c0 + CONV_CH], cv_ps,
                                     resid_sb[:, c0:c0 + CONV_CH])
                pstate["rc"] += 1
        # ---- flash attention main loop ----------------------------------
        for qc in range(NQC):
            q0 = qc * QCHUNK
            qn = min(QCHUNK, QPAD - q0)
            U_ps = UP.tile([128, qn], F32, tag="U", name="U_ps",
                           padded_shape=[128, QCHUNK])
            for t in range(KB // KB_GRP):
                S_ps = SP.tile([128, KB_GRP, qn], F32, tag="S", name="S_ps",
                               padded_shape=[128, KB_GRP, QCHUNK])
                for j in range(KB_GRP):
                    kb = t * KB_GRP + j
                    nc.tensor.matmul(
                        S_ps[:, j, :],
                        lhsT=xk_sb[:, kb * 128:(kb + 1) * 128] if MM1_FLAT
                        else xk_sb[32 * j:32 * j + C, kb * 128:(kb + 1) * 128],
                        rhs=xq_sb[:, q0:q0 + qn] if MM1_FLAT
                        else xq_sb[32 * j:32 * j + C, q0:q0 + qn],
                        start=True, stop=True,
                    )
                e_sb = EPl.tile([128, KB_GRP, qn], BF16, tag="e", name="e_sb",
                                padded_shape=[128, KB_GRP, QCHUNK])
                nc.scalar.activation(e_sb, S_ps, EXP, scale=INV_SQRT_C)
                for j in range(KB_GRP):
                    kb = t * KB_GRP + j
                    nc.tensor.matmul(
                        U_ps[32 * j:32 * j + 4, 0:qn],
                        lhsT=v4_sb[:, kb * 4:kb * 4 + 4],
                        rhs=e_sb[:, j, :],
                        start=(t == 0), stop=(t == KB // KB_GRP - 1),
                        skip_group_check=True,
                    )

            # softmax division via PE transposes (partition-aligned only)
            U_sb = WK.tile([128, qn], TRS, tag="Usb", name="U_sb",
                           padded_shape=[128, QCHUNK])
            nc.vector.tensor_copy(U_sb, U_ps)
            for c in range((qn + 127) // 128):
                cw = min(128, qn - c * 128)
                # last qc: main loop is done, the S slots are free -- use
                # them so the exposed tail divide double-buffers and the
                # TP slot stays free for the conv chunks
                TPt = (SP if qc == NQC - 1 else TP).tile(
                    [128, 132], F32,
                    tag="S" if qc == NQC - 1 else "T", name="TPt",
                    padded_shape=None if qc == NQC - 1 else [128, QCHUNK])
                Ut = TPt[0:cw, 0:4]
                # transpose-and-sum the three column-group partials
                for j in range(KB_GRP):
                    nc.tensor.matmul(
                        Ut,
                        lhsT=U_sb[32 * j:32 * j + 4, c * 128:c * 128 + cw],
                        rhs=eye4x_sb[32 * j:32 * j + 4, :],
                        start=(j == 0), stop=(j == KB_GRP - 1),
                        skip_group_check=True,
                    )
                r_sb = SM.tile([128, 1], F32, tag="r", name="r_sb")
                nc.vector.reciprocal(r_sb[0:cw, :], Ut[:, 3:4])
                Ft_sb = SM.tile([128, 4], TRS, tag="Ft", name="Ft_sb")
                nc.vector.tensor_scalar_mul(Ft_sb[0:cw, :], Ut, r_sb[0:cw, :])
                Fb = TPt[0:4, 4:4 + cw]
                nc.tensor.matmul(Fb, lhsT=Ft_sb[0:cw, :],
                                 rhs=id128_sb[0:cw, 0:cw],
                                 start=True, stop=True)
                nc.vector.tensor_copy(F_sb[:, q0 + c * 128:q0 + c * 128 + cw],
                                      Fb[0:3, :])
            nc.vector.tensor_mul(F_m[:, q0:q0 + qn],
                                 F_sb[:, q0:q0 + qn],
                                 mask_sb[:, q0:q0 + qn])
            if qc >= 2:
                emit_pipeline(min(HALO_ROWS, (q0 + qn) // W))


        # ---- tail: (conv emitted per-qc above) ---------------------------
        out_pad_r = out_pad.rearrange("p (r w) -> p r w", w=WP)
        nc.sync.dma_start(out=out[:].rearrange("p (r w) -> p r w", w=W),
                          in_=out_pad_r[:, :, 1:1 + W])

    nc.compile()
    return nc


_CACHE: dict = {}


def _get_nc() -> bass.Bass:
    if "nc" not in _CACHE:
        _CACHE["nc"] = build_nc()
    return _CACHE["nc"]


def make_in_maps(input, conv_w, conv_b):
    input = np.ascontiguousarray(np.asarray(input, dtype=np.float32))
    conv_w = np.asarray(conv_w, dtype=np.float32)
    conv_b = np.asarray(conv_b, dtype=np.float32)
    x = input.reshape(B, C, N)

    # conv weights: wmat[3s+i, o] = conv_w[o, i, dy+1, dx+1] for shift s
    wmat = np.empty((27, C), np.float32)
    for s, (dy, dx) in enumerate(SHIFTS):
        wmat[3 * s:3 * s + 3, :] = conv_w[:, :, dy + 1, dx + 1].T  # [i, o]
    wmat = wmat.astype(ml_dtypes.bfloat16)
    eye4x = np.zeros((128, 4), np.float32)
    for j in range(4):
        eye4x[32 * j:32 * j + 4, :] = np.eye(4, dtype=np.float32)
    id128 = np.eye(128, dtype=np.float32)

    in_maps = []
    for b in range(B):
        xb = x[b]
        xk_pad = np.zeros((128, N), np.float32)
        xk_pad[:C] = xb
        v4 = np.ones((128, KB, 4), np.float32)
        v4[:, :, :3] = xb.reshape(C, KB, 128).transpose(2, 1, 0)
        v4 = v4.reshape(128, KB * 4).astype(ml_dtypes.bfloat16)
        for j in range(BANDS):
            r0 = j * BAND_ROWS
            lo = r0 - 1
            xqb = np.zeros((C, HALO_ROWS, W), np.float32)
            mk = np.zeros((C, HALO_ROWS, W), np.float32)
            a = max(lo, 0)
            bb = min(lo + HALO_ROWS, H)
            xqb[:, a - lo:bb - lo, :] = input[b][:, a:bb, :]
            mk[:, a - lo:bb - lo, :] = 1.0
            xq_pad = np.zeros((128, QPAD), np.float32)
            xq_pad[:C, :QN] = xqb.reshape(C, QN)
            mk_pad = np.zeros((C, QPAD), np.float32)
            mk_pad[:, :QN] = mk.reshape(C, QN)
            residb = np.zeros((C, BAND_ROWS, WP), np.float32)
            residb[:, :, 1:1 + W] = (input[b][:, r0:r0 + BAND_ROWS, :]
                                     + conv_b[:, None, None])
            in_maps.append({
                "xk": xk_pad,
                "xq": xq_pad,
                "v4": v4,
                "mask": mk_pad,
                "resid": residb.reshape(C, CONVN),
                "wmat": wmat,
                "eye4x": eye4x,
                "id128": id128,
            })
    return in_maps


def run(input, conv_w, conv_b, trace=False, **spmd_kwargs):
    in_maps = make_in_maps(input, conv_w, conv_b)
    res = run_bass_kernel_spmd(_get_nc(), in_maps, list(range(2 * BANDS)),
                               trace=trace, **spmd_kwargs)
    out = np.empty((B, C, H, W), np.float32)
    for b in range(B):
        for j in range(BANDS):
            band = res.results[b * BANDS + j]["out"]
            out[b, :, j * BAND_ROWS:(j + 1) * BAND_ROWS, :] = (
                band.reshape(C, BAND_ROWS, W))
    return out, res


def kernel(input, conv_w, conv_b) -> np.ndarray:
    out, _ = run(input, conv_w, conv_b)
    return out
